# revision 1
# baseline (speedup 1.0000x reference)
"""Trainium2 Bass kernel for nn_L1OutUB (L1-out upper bound contrastive loss).

Math: the reference builds a [B,B,B] tensor `inpt[a,i,j] = all_probs[i,j] +
(-20 if a==i else 0)` and logsumexps over `a`.  That logsumexp is exactly
`all_probs[i,j] + log(B-1+e^-20)`, so

    result = mean(positive) - mean(all_probs) - log1p(e^-20 / (B-1))

and `sum_j all_probs[i,j]` collapses onto per-column moments of y:

    sum_j (y[j,d]-mu[i,d])^2 = S2[d] - 2*M1[d]*mu[i,d] + B*mu[i,d]^2
    with S2[d] = sum_j y[j,d]^2,  M1[d] = sum_j y[j,d].

The -0.5*logvar terms of positive/all_probs cancel exactly in the final
difference, leaving two fused multiply-reduce passes per core.

Sharding: rows of x across 8 cores (64 rows each); every core gets the full
(row-rotated) y so its matched rows sit at positions 0:64 and the global
column moments are unchanged by the rotation.  Host sums the 8 scalar
partials (the "all-reduce").

Layout/overlap notes:
  - x split across the two HWDGE queues (SP + ACT) to halve its landing time;
    weight blobs (2 packed DMAs instead of 8) go right behind it.
  - y column moments computed on PE: m1x2 = y.T @ twos, s2 = (y*y).T @ ones,
    accumulated over 4 row-tiles; avoids ACT Square table load + DVE reduce
    over [128,512].
  - both MLPs' first layers share one matmul chain (w1_mu|w1_lv packed to a
    [128,6,16] blob -> hboth [16,64]); w2_lv sits at partitions 8:16 so the
    second-layer matmuls read hboth slices at matching base partitions.
  - positive-branch elementwise chain runs on GPSIMD in parallel with the
    all-pairs chain on DVE.
"""

import numpy as np

import concourse.bacc as bacc
import concourse.tile as tile
from concourse import mybir
from concourse.masks import make_identity

F32 = mybir.dt.float32
AF = mybir.ActivationFunctionType
ALU = mybir.AluOpType

B, X_DIM, Y_DIM, HID = 512, 768, 128, 8
N_CORES = 8
R = B // N_CORES          # rows per core = 64
XC = X_DIM // 128         # x feature chunks = 6
XH = X_DIM // 2

_CACHE = {}


def _build():
    nc = bacc.Bacc("TRN2", target_bir_lowering=False, debug=False,
                   num_devices=N_CORES)

    x_d = nc.dram_tensor("x", [R, X_DIM], F32, kind="ExternalInput")
    y_d = nc.dram_tensor("y", [B, Y_DIM], F32, kind="ExternalInput")
    # wb1: [128, 242] = w1 chunks ([128,6,40]: w1_mu_k at +0:8,
    #      w1_lv_k at +32:40), b2_mu (col 240), b2_lv (col 241)
    wb1_d = nc.dram_tensor("wb1", [128, 242], F32, kind="ExternalInput")
    # wb2: [40, 258] = w2_mu at [0:8, 0:128], w2_lv at [32:40, 128:256]
    #      (matmul base partition must be 0/32/64), b1 in col 256
    #      (rows 0:8 = b1_mu, rows 32:40 = b1_lv)
    wb2_d = nc.dram_tensor("wb2", [40, 258], F32, kind="ExternalInput")
    out_d = nc.dram_tensor("out", [1, 1], F32, kind="ExternalOutput")

    with tile.TileContext(nc) as tc:
        with (
            tc.tile_pool(name="sb", bufs=1) as sb,
            tc.tile_pool(name="ps", bufs=1, space="PSUM") as ps,
        ):
            # ---- loads: x halves on the two HWDGE queues, then blobs, then y
            x_s = sb.tile([R, X_DIM], F32, tag="x")
            nc.sync.dma_start(out=x_s[:, 0:XH], in_=x_d[:, 0:XH])
            nc.scalar.dma_start(out=x_s[:, XH:X_DIM], in_=x_d[:, XH:X_DIM])
            wb2_s = sb.tile([40, 258], F32, tag="wb2")
            nc.sync.dma_start(out=wb2_s[:], in_=wb2_d[:])
            wb1_s = sb.tile([128, 242], F32, tag="wb1")
            nc.scalar.dma_start(out=wb1_s[:], in_=wb1_d[:])
            y_s = sb.tile([128, 4, 128], F32, tag="y")
            y_r = y_d.rearrange("(t p) c -> p t c", p=128)
            nc.sync.dma_start(out=y_s[:, 0:2, :], in_=y_r[:, 0:2, :])
            nc.scalar.dma_start(out=y_s[:, 2:4, :], in_=y_r[:, 2:4, :])

            ident = sb.tile([R, R], F32, tag="ident")
            make_identity(nc, ident[:])
            ones_s = sb.tile([128, 2], F32, tag="ones")   # col 0 = 1.0, col 1 = 2.0
            nc.vector.memset(ones_s[:, 0:1], 1.0)
            nc.vector.memset(ones_s[:, 1:2], 2.0)

            # ---- y column moments on PE: m1x2 = y.T @ 2, s2 = (y*y).T @ 1
            ysq_s = sb.tile([128, 4, 128], F32, tag="ysq")
            nc.vector.tensor_mul(ysq_s[:], y_s[:], y_s[:])
            st_p = ps.tile([128, 2], F32, tag="st")
            for t in range(4):
                nc.tensor.matmul(st_p[:, 0:1], y_s[:, t, :], ones_s[:, 1:2],
                                 start=(t == 0), stop=(t == 3))
            for t in range(4):
                nc.tensor.matmul(st_p[:, 1:2], ysq_s[:, t, :], ones_s[:, 0:1],
                                 start=(t == 0), stop=(t == 3))
            st_s = sb.tile([128, 2], F32, tag="sts")
            nc.vector.tensor_copy(out=st_s[:], in_=st_p[:])

            # ---- transpose of this core's matched y rows (rotation put them
            # at rows 0:64 = tile 0, partitions 0:64)
            ycT_p = ps.tile([Y_DIM, R], F32, tag="ycT")
            nc.tensor.transpose(ycT_p[:], y_s[0:R, 0, :], ident[:])
            ycT_s = sb.tile([Y_DIM, R], F32, tag="ycTs")
            nc.vector.tensor_copy(out=ycT_s[:], in_=ycT_p[:])

            # ---- transpose x -> xT chunks [128, XC*64] ----
            xT_p = ps.tile([128, XC * R], F32, tag="xT")
            for k in range(XC):
                nc.tensor.transpose(xT_p[:, k * R:(k + 1) * R],
                                    x_s[:, k * 128:(k + 1) * 128], ident[:])
            xT_s = sb.tile([128, XC * R], F32, tag="xTs")
            nc.vector.tensor_copy(out=xT_s[:], in_=xT_p[:])

            # ---- MLP layer 1 (both nets fused): hboth = relu(w1.T @ xT + b1)
            hb_p = ps.tile([40, R], F32, tag="hb")
            for k in range(XC):
                nc.tensor.matmul(hb_p[:], wb1_s[:, k * 40:(k + 1) * 40],
                                 xT_s[:, k * R:(k + 1) * R],
                                 start=(k == 0), stop=(k == XC - 1))
            hb_s = sb.tile([40, R], F32, tag="hbs")
            nc.scalar.activation(out=hb_s[:], in_=hb_p[:], func=AF.Relu,
                                 bias=wb2_s[:, 256:257])

            # ---- MLP layer 2: muT = w2m.T @ hm + b2m ; lvT = tanh(.) ----
            mu_p = ps.tile([Y_DIM, R], F32, tag="mup")
            lv_p = ps.tile([Y_DIM, R], F32, tag="lvp")
            nc.tensor.matmul(mu_p[:], wb2_s[0:8, 0:128], hb_s[0:8, :],
                             start=True, stop=True)
            nc.tensor.matmul(lv_p[:], wb2_s[32:40, 128:256], hb_s[32:40, :],
                             start=True, stop=True)
            mu_s = sb.tile([Y_DIM, R], F32, tag="mus")
            lv_s = sb.tile([Y_DIM, R], F32, tag="lvs")
            inv_s = sb.tile([Y_DIM, R], F32, tag="invs")
            nc.scalar.activation(out=mu_s[:], in_=mu_p[:], func=AF.Identity,
                                 bias=wb1_s[:, 240:241])
            nc.scalar.activation(out=lv_s[:], in_=lv_p[:], func=AF.Tanh,
                                 bias=wb1_s[:, 241:242])
            nc.scalar.activation(out=inv_s[:], in_=lv_s[:], func=AF.Exp,
                                 scale=-1.0)

            # ---- positive branch on GPSIMD: a = -(0.5/B) * (mu - yc)^2 ----
            d_s = sb.tile([Y_DIM, R], F32, tag="ds")
            nc.gpsimd.tensor_sub(d_s[:], mu_s[:], ycT_s[:])
            dsq_s = sb.tile([Y_DIM, R], F32, tag="dsq")
            nc.gpsimd.tensor_mul(dsq_s[:], d_s[:], d_s[:])
            a_s = sb.tile([Y_DIM, R], F32, tag="as")
            nc.gpsimd.tensor_scalar_mul(a_s[:], dsq_s[:], -0.5 / B)

            # ---- all-pairs branch on DVE: b = (0.5/B^2)*(B*mu^2-2*M1*mu+S2)
            t_s = sb.tile([Y_DIM, R], F32, tag="ts")
            nc.vector.tensor_scalar(out=t_s[:], in0=mu_s[:], scalar1=float(B),
                                    scalar2=st_s[:, 0:1], op0=ALU.mult,
                                    op1=ALU.subtract)
            q_s = sb.tile([Y_DIM, R], F32, tag="qs")
            nc.vector.tensor_mul(q_s[:], t_s[:], mu_s[:])
            nc.vector.tensor_scalar_add(q_s[:], q_s[:], st_s[:, 1:2])
            b_s = sb.tile([Y_DIM, R], F32, tag="bs")
            nc.vector.tensor_scalar_mul(b_s[:], q_s[:], 0.5 / (B * B))

            # ---- combine, weight by inv_var, reduce ----
            c_s = sb.tile([Y_DIM, R], F32, tag="cs")
            nc.vector.tensor_add(c_s[:], a_s[:], b_s[:])
            w_s = sb.tile([Y_DIM, R], F32, tag="ws")
            nc.vector.tensor_mul(w_s[:], c_s[:], inv_s[:])
            tot_s = sb.tile([Y_DIM, 1], F32, tag="tot")
            nc.vector.tensor_reduce(out=tot_s[:], in_=w_s[:],
                                    axis=mybir.AxisListType.X, op=ALU.add)
            res_p = ps.tile([1, 1], F32, tag="res")
            nc.tensor.matmul(res_p[:], tot_s[:], ones_s[:, 0:1],
                             start=True, stop=True)
            res_s = sb.tile([1, 1], F32, tag="ress")
            nc.vector.tensor_copy(out=res_s[:], in_=res_p[:])
            nc.sync.dma_start(out=out_d[:], in_=res_s[:])

    nc.compile()
    return nc


def _get_nc():
    if "nc" not in _CACHE:
        _CACHE["nc"] = _build()
    return _CACHE["nc"]


def _pack_weights(w1_mu, b1_mu, w2_mu, b2_mu, w1_lv, b1_lv, w2_lv, b2_lv):
    f = np.float32
    wb1 = np.zeros((128, 242), f)
    w1m = np.asarray(w1_mu, f).reshape(XC, 128, HID)
    w1l = np.asarray(w1_lv, f).reshape(XC, 128, HID)
    for k in range(XC):
        wb1[:, k * 40:k * 40 + 8] = w1m[k]
        wb1[:, k * 40 + 32:k * 40 + 40] = w1l[k]
    wb1[:, 240] = np.asarray(b2_mu, f)
    wb1[:, 241] = np.asarray(b2_lv, f)
    wb2 = np.zeros((40, 258), f)
    wb2[0:8, 0:128] = np.asarray(w2_mu, f)
    wb2[32:40, 128:256] = np.asarray(w2_lv, f)
    wb2[0:8, 256] = np.asarray(b1_mu, f)
    wb2[32:40, 256] = np.asarray(b1_lv, f)
    return wb1, wb2


def kernel(x_samples, y_samples, w1_mu, b1_mu, w2_mu, b2_mu,
           w1_lv, b1_lv, w2_lv, b2_lv, **profile_kwargs):
    from concourse import bass_utils

    f = np.float32
    y = np.ascontiguousarray(y_samples, f)
    wb1, wb2 = _pack_weights(w1_mu, b1_mu, w2_mu, b2_mu,
                             w1_lv, b1_lv, w2_lv, b2_lv)
    in_maps = []
    for c in range(N_CORES):
        in_maps.append({
            "x": np.ascontiguousarray(x_samples[c * R:(c + 1) * R], f),
            "y": np.ascontiguousarray(np.roll(y, -c * R, axis=0)),
            "wb1": wb1,
            "wb2": wb2,
        })

    nc = _get_nc()
    res = bass_utils.run_bass_kernel_spmd(
        nc, in_maps, core_ids=list(range(N_CORES)), **profile_kwargs
    )
    total = sum(float(m["out"][0, 0]) for m in res.results)
    total -= np.log1p(np.exp(-20.0) / (B - 1))
    out = np.array(total, dtype=np.float32)
    if profile_kwargs:
        return out, res
    return out



# revision 5
# speedup vs baseline: 1.2595x; 1.2595x over previous
"""Trainium2 Bass kernel for nn_L1OutUB (L1-out upper bound contrastive loss).

Math: the reference builds a [B,B,B] tensor `inpt[a,i,j] = all_probs[i,j] +
(-20 if a==i else 0)` and logsumexps over `a`.  That logsumexp is exactly
`all_probs[i,j] + log(B-1+e^-20)`, so

    result = mean(positive) - mean(all_probs) - log1p(e^-20 / (B-1))

`sum_j all_probs[i,j]` collapses onto per-column moments of y, and the
-0.5*logvar terms cancel between positive and negative.  Expanding the
positive-branch square as well, every remaining term is a contraction of
per-core row sums:

    A[d]  = sum_i iv[i,d]          C[d]  = sum_i mu[i,d] iv[i,d]
    D     = sum_{i,d} mu^2 iv      U1    = sum_{i,d} mu y iv   (matched y)
    U2    = sum_{i,d} y^2 iv       S2[d] = sum_j y[j,d]^2      M1[d] = sum_j y[j,d]

    P     = D - 2 U1 + U2          (positive-branch quadratic)
    result = -(P/2B) + (1/2B^2)(S2.A - 2 M1.C + B D) - log1p(e^-20/(B-1))

Sharding: rows of x/y across 8 cores (64 rows each); each core emits its
partial vectors [128, 7] = (A, C, Dv, U1v, U2v, S2, M1); the host sums the
8 partials and does three dot products (the "all-reduce").

Device-side structure per core (layout [d=128 partitions, r=64 free]):
  - host pre-transposes x (xT chunks) and y (yT), packs both MLPs' layer-1
    weights into one lhsT blob with a zero column whose relu(0 + bias=1)
    manufactures the ones-row that folds the layer-2 bias into the matmul.
  - PE: 6 accumulating L1 matmuls -> relu -> 2 L2 matmuls (mu | z), biases
    ride the ones-row, so mu/z land in PSUM fully biased.
  - Scalar: relu, tanh(scale=-1), exp  (iv = exp(-tanh(z)); logvar itself
    cancels).  The exp's accum_out yields A for free.
  - DVE: tensor_tensor_reduce ops fuse each product with its row-sum.
  - No transposes, no GpSimd, no collectives; one [128,7] output DMA.
"""

import numpy as np

import concourse.bacc as bacc
import concourse.tile as tile
from concourse import mybir

F32 = mybir.dt.float32
AF = mybir.ActivationFunctionType
ALU = mybir.AluOpType
AX = mybir.AxisListType

B, X_DIM, Y_DIM, HID = 512, 768, 128, 8
N_CORES = 8
R = B // N_CORES          # rows per core = 64
XC = X_DIM // 128         # x feature chunks = 6
WCOL = 41                 # packed L1 lhsT cols: 0:8 mu, 8 zero, 32:40 lv, 40 zero
CHUNK = WCOL + R          # per-chunk blob cols = 105

_CACHE = {}


def _build():
    nc = bacc.Bacc("TRN2", target_bir_lowering=False, debug=False,
                   num_devices=N_CORES)

    # a1/a2: 3 chunks each of [w1p_k [128,41] | xT_k [128,64]]
    a1_d = nc.dram_tensor("a1", [128, 3 * CHUNK], F32, kind="ExternalInput")
    a2_d = nc.dram_tensor("a2", [128, 3 * CHUNK], F32, kind="ExternalInput")
    # yb: cols 0:64 = yT (this core's y slice, transposed), col 64 = b1vec
    #     (rows 0:8 b1_mu, row 8 = 1.0, rows 32:40 b1_lv, row 40 = 1.0)
    yb_d = nc.dram_tensor("yb", [128, 65], F32, kind="ExternalInput")
    # w2: rows 0:8 w2_mu, row 8 b2_mu; rows 32:40 w2_lv, row 40 b2_lv
    w2_d = nc.dram_tensor("w2", [WCOL, 128], F32, kind="ExternalInput")
    # out columns: A, C, Dv, U1v, U2v, S2, M1
    out_d = nc.dram_tensor("out", [128, 7], F32, kind="ExternalOutput")

    with tile.TileContext(nc) as tc:
        with (
            tc.tile_pool(name="sb", bufs=1) as sb,
            tc.tile_pool(name="ps", bufs=1, space="PSUM") as ps,
        ):
            # ---- loads: weights+x on sync ring, y+w2 on scalar ring ----
            yb_s = sb.tile([128, 65], F32, tag="yb")
            nc.scalar.dma_start(out=yb_s[:], in_=yb_d[:])
            w2_s = sb.tile([WCOL, 128], F32, tag="w2")
            nc.scalar.dma_start(out=w2_s[:], in_=w2_d[:])
            a1_s = sb.tile([128, 3 * CHUNK], F32, tag="a1")
            nc.sync.dma_start(out=a1_s[:], in_=a1_d[:])
            a2_s = sb.tile([128, 3 * CHUNK], F32, tag="a2")
            nc.sync.dma_start(out=a2_s[:], in_=a2_d[:])

            yT = yb_s[:, 0:R]
            outv = sb.tile([128, 7], F32, tag="outv")
            ysq_s = sb.tile([128, R], F32, tag="ysq")

            # ---- early y moments (only need this core's slice) ----
            nc.vector.tensor_mul(ysq_s[:], yT, yT)
            nc.vector.tensor_reduce(out=outv[:, 5:6], in_=ysq_s[:],
                                    axis=AX.X, op=ALU.add)
            nc.vector.tensor_reduce(out=outv[:, 6:7], in_=yT,
                                    axis=AX.X, op=ALU.add)

            # ---- L1 (both nets fused): ps1 = w1p.T @ xT over 6 chunks ----
            ps1 = ps.tile([WCOL, R], F32, tag="ps1")
            for k in range(XC):
                src = a1_s if k < 3 else a2_s
                j = (k % 3) * CHUNK
                nc.tensor.matmul(ps1[:], src[:, j:j + WCOL],
                                 src[:, j + WCOL:j + CHUNK],
                                 start=(k == 0), stop=(k == XC - 1))
            hb_s = sb.tile([WCOL, R], F32, tag="hb")
            nc.scalar.activation(out=hb_s[:], in_=ps1[:], func=AF.Relu,
                                 bias=yb_s[0:WCOL, 64:65])

            # ---- L2: mu and z with bias folded in via the ones-rows ----
            ps2m = ps.tile([128, R], F32, tag="ps2m")
            ps2l = ps.tile([128, R], F32, tag="ps2l")
            nc.tensor.matmul(ps2m[:], w2_s[0:9, :], hb_s[0:9, :],
                             start=True, stop=True)
            nc.tensor.matmul(ps2l[:], w2_s[32:41, :], hb_s[32:41, :],
                             start=True, stop=True)

            # ---- iv = exp(-tanh(z)); exp's accum gives A = sum_r iv ----
            th_s = sb.tile([128, R], F32, tag="th")
            nc.scalar.activation(out=th_s[:], in_=ps2l[:], func=AF.Tanh,
                                 scale=-1.0)
            iv_s = sb.tile([128, R], F32, tag="iv")
            nc.scalar.activation(out=iv_s[:], in_=th_s[:], func=AF.Exp,
                                 accum_out=outv[:, 0:1])

            # ---- products on DVE; row-sums split DVE/Scalar ----
            t1_s = sb.tile([128, R], F32, tag="t1")
            nc.vector.tensor_mul(t1_s[:], ps2m[:], iv_s[:])
            t2_s = sb.tile([128, R], F32, tag="t2")
            nc.vector.tensor_mul(t2_s[:], t1_s[:], ps2m[:])
            u1_s = sb.tile([128, R], F32, tag="u1")
            nc.vector.tensor_mul(u1_s[:], t1_s[:], yT)
            u2_s = sb.tile([128, R], F32, tag="u2")
            nc.vector.tensor_mul(u2_s[:], ysq_s[:], iv_s[:])
            # C, U1 ride scalar-engine ACT accum (th_s is dead, reuse as sink)
            nc.scalar.activation(out=th_s[:], in_=t1_s[:], func=AF.Identity,
                                 accum_out=outv[:, 1:2])
            nc.scalar.activation(out=th_s[:], in_=u1_s[:], func=AF.Identity,
                                 accum_out=outv[:, 3:4])
            nc.vector.tensor_reduce(out=outv[:, 2:3], in_=t2_s[:],
                                    axis=AX.X, op=ALU.add)
            nc.vector.tensor_reduce(out=outv[:, 4:5], in_=u2_s[:],
                                    axis=AX.X, op=ALU.add)

            nc.sync.dma_start(out=out_d[:], in_=outv[:])

    nc.compile()
    return nc


def _get_nc():
    if "nc" not in _CACHE:
        _CACHE["nc"] = _build()
    return _CACHE["nc"]


def _pack(x_samples, y_samples, w1_mu, b1_mu, w2_mu, b2_mu,
          w1_lv, b1_lv, w2_lv, b2_lv):
    f = np.float32
    w1m = np.asarray(w1_mu, f).reshape(XC, 128, HID)
    w1l = np.asarray(w1_lv, f).reshape(XC, 128, HID)
    w1p = np.zeros((XC, 128, WCOL), f)
    w1p[:, :, 0:8] = w1m
    w1p[:, :, 32:40] = w1l

    b1vec = np.zeros((128, 1), f)
    b1vec[0:8, 0] = np.asarray(b1_mu, f)
    b1vec[8, 0] = 1.0
    b1vec[32:40, 0] = np.asarray(b1_lv, f)
    b1vec[40, 0] = 1.0

    w2 = np.zeros((WCOL, 128), f)
    w2[0:8] = np.asarray(w2_mu, f)
    w2[8] = np.asarray(b2_mu, f)
    w2[32:40] = np.asarray(w2_lv, f)
    w2[40] = np.asarray(b2_lv, f)

    x = np.asarray(x_samples, f)
    y = np.asarray(y_samples, f)
    in_maps = []
    for c in range(N_CORES):
        xs = x[c * R:(c + 1) * R]                       # [64, 768]
        xT = xs.reshape(R, XC, 128).transpose(1, 2, 0)  # [6, 128, 64]
        chunks = np.empty((XC, 128, CHUNK), f)
        chunks[:, :, 0:WCOL] = w1p
        chunks[:, :, WCOL:CHUNK] = xT
        a1 = np.ascontiguousarray(
            chunks[0:3].transpose(1, 0, 2).reshape(128, 3 * CHUNK))
        a2 = np.ascontiguousarray(
            chunks[3:6].transpose(1, 0, 2).reshape(128, 3 * CHUNK))
        yb = np.empty((128, 65), f)
        yb[:, 0:R] = y[c * R:(c + 1) * R].T
        yb[:, 64:65] = b1vec
        in_maps.append({"a1": a1, "a2": a2, "yb": yb, "w2": w2})
    return in_maps


def kernel(x_samples, y_samples, w1_mu, b1_mu, w2_mu, b2_mu,
           w1_lv, b1_lv, w2_lv, b2_lv, **profile_kwargs):
    from concourse import bass_utils

    in_maps = _pack(x_samples, y_samples, w1_mu, b1_mu, w2_mu, b2_mu,
                    w1_lv, b1_lv, w2_lv, b2_lv)
    nc = _get_nc()
    res = bass_utils.run_bass_kernel_spmd(
        nc, in_maps, core_ids=list(range(N_CORES)), **profile_kwargs
    )
    acc = np.zeros((128, 7), np.float64)
    for m in res.results:
        acc += m["out"].astype(np.float64)
    A, C, Dv, U1v, U2v, S2, M1 = (acc[:, j] for j in range(7))
    D = Dv.sum()
    P = D - 2.0 * U1v.sum() + U2v.sum()
    neg = (S2 @ A - 2.0 * (M1 @ C) + B * D) / (2.0 * B * B)
    total = -P / (2.0 * B) + neg - np.log1p(np.exp(-20.0) / (B - 1.0))
    out = np.array(total, dtype=np.float32)
    if profile_kwargs:
        return out, res
    return out


# revision 6
# speedup vs baseline: 1.3365x; 1.0611x over previous
"""Trainium2 Bass kernel for nn_L1OutUB (L1-out upper bound contrastive loss).

Math: the reference builds a [B,B,B] tensor `inpt[a,i,j] = all_probs[i,j] +
(-20 if a==i else 0)` and logsumexps over `a`.  That logsumexp is exactly
`all_probs[i,j] + log(B-1+e^-20)`, so

    result = mean(positive) - mean(all_probs) - log1p(e^-20 / (B-1))

`sum_j all_probs[i,j]` collapses onto per-column moments of y, and the
-0.5*logvar terms cancel between positive and negative.  Expanding the
positive-branch square as well, every remaining term is a contraction of
per-core row sums:

    A[d]  = sum_i iv[i,d]          C[d]  = sum_i mu[i,d] iv[i,d]
    D     = sum_{i,d} mu^2 iv      U1    = sum_{i,d} mu y iv   (matched y)
    U2    = sum_{i,d} y^2 iv       S2[d] = sum_j y[j,d]^2      M1[d] = sum_j y[j,d]

    P     = D - 2 U1 + U2          (positive-branch quadratic)
    result = -(P/2B) + (1/2B^2)(S2.A - 2 M1.C + B D) - log1p(e^-20/(B-1))

Sharding: rows of x/y across 8 cores (64 rows each); each core emits its
partial vectors [128, 7] = (A, C, Dv, U1v, U2v, S2, M1); the host sums the
8 partials and does three dot products (the "all-reduce").

Device-side structure per core (layout [d=128 partitions, r=64 free]):
  - host pre-transposes x (xT chunks) and y (yT), packs both MLPs' layer-1
    weights into one dense [128,18] lhsT per chunk whose two zero columns,
    via relu(0 + bias=1), manufacture ones-rows that fold the layer-2
    biases into the matmuls.  psum M=18 <= 32 so L1 matmuls don't split.
  - PE: 6 accumulating L1 matmuls -> relu -> 2 L2 matmuls whose lhsT are
    zero-padded to base partition 0 ([18,128] each), writing mu | z into
    one [128,128] psum; biases ride the ones-rows.
  - Scalar: relu, tanh(scale=-1), exp+accum (iv = exp(-tanh(z)); logvar
    itself cancels; the exp accumulator yields A for free).
  - DVE: 4 products + cheap 128ns row-reduces; y moments done early.
  - No transposes, no GpSimd, no collectives; one [128,7] output DMA.
"""

import numpy as np

import concourse.bacc as bacc
import concourse.tile as tile
from concourse import mybir

F32 = mybir.dt.float32
AF = mybir.ActivationFunctionType
ALU = mybir.AluOpType
AX = mybir.AxisListType

B, X_DIM, Y_DIM, HID = 512, 768, 128, 8
N_CORES = 8
R = B // N_CORES          # rows per core = 64
XC = X_DIM // 128         # x feature chunks = 6
WCOL = 18                 # packed L1 lhsT cols: 0:8 mu, 8 zero, 9:17 lv, 17 zero
CHUNK = WCOL + R          # per-chunk blob cols = 82

_CACHE = {}


def _build():
    nc = bacc.Bacc("TRN2", target_bir_lowering=False, debug=False,
                   num_devices=N_CORES)

    # a1/a2: 3 chunks each of [w1p_k [128,18] | xT_k [128,64]]
    a1_d = nc.dram_tensor("a1", [128, 3 * CHUNK], F32, kind="ExternalInput")
    a2_d = nc.dram_tensor("a2", [128, 3 * CHUNK], F32, kind="ExternalInput")
    # yb: cols 0:64 = yT (this core's y slice, transposed), col 64 = b1vec
    #     (rows 0:8 b1_mu, row 8 = 1.0, rows 9:17 b1_lv, row 17 = 1.0)
    yb_d = nc.dram_tensor("yb", [128, 65], F32, kind="ExternalInput")
    # w2: cols 0:128 = mu block (rows 0:8 w2_mu, row 8 b2_mu, rows 9:18 zero)
    #     cols 128:256 = lv block (rows 0:9 zero, rows 9:17 w2_lv, row 17 b2_lv)
    w2_d = nc.dram_tensor("w2", [WCOL, 256], F32, kind="ExternalInput")
    # out columns: A, C, Dv, U1v, U2v, S2, M1
    out_d = nc.dram_tensor("out", [128, 7], F32, kind="ExternalOutput")

    with tile.TileContext(nc) as tc:
        with (
            tc.tile_pool(name="sb", bufs=1) as sb,
            tc.tile_pool(name="ps", bufs=1, space="PSUM") as ps,
        ):
            # ---- loads: x-bearing blobs first on both rings ----
            a1_s = sb.tile([128, 3 * CHUNK], F32, tag="a1")
            nc.sync.dma_start(out=a1_s[:], in_=a1_d[:])
            a2_s = sb.tile([128, 3 * CHUNK], F32, tag="a2")
            nc.scalar.dma_start(out=a2_s[:], in_=a2_d[:])
            yb_s = sb.tile([128, 65], F32, tag="yb")
            nc.sync.dma_start(out=yb_s[:], in_=yb_d[:])
            w2_s = sb.tile([WCOL, 256], F32, tag="w2")
            nc.scalar.dma_start(out=w2_s[:], in_=w2_d[:])

            yT = yb_s[:, 0:R]
            outv = sb.tile([128, 7], F32, tag="outv")
            ysq_s = sb.tile([128, R], F32, tag="ysq")

            # ---- early y moments (only need this core's slice) ----
            nc.vector.tensor_mul(ysq_s[:], yT, yT)
            nc.vector.tensor_reduce(out=outv[:, 5:6], in_=ysq_s[:],
                                    axis=AX.X, op=ALU.add)
            nc.vector.tensor_reduce(out=outv[:, 6:7], in_=yT,
                                    axis=AX.X, op=ALU.add)

            # ---- L1 (both nets fused): ps1 = w1p.T @ xT over 6 chunks ----
            ps1 = ps.tile([WCOL, R], F32, tag="ps1")
            for k in range(XC):
                src = a1_s if k < 3 else a2_s
                j = (k % 3) * CHUNK
                nc.tensor.matmul(ps1[:], src[:, j:j + WCOL],
                                 src[:, j + WCOL:j + CHUNK],
                                 start=(k == 0), stop=(k == XC - 1))
            hb_s = sb.tile([WCOL, R], F32, tag="hb")
            nc.scalar.activation(out=hb_s[:], in_=ps1[:], func=AF.Relu,
                                 bias=yb_s[0:WCOL, 64:65])

            # ---- L2 into one psum: cols 64:128 = z first, then 0:64 = mu ----
            ps2 = ps.tile([128, 2 * R], F32, tag="ps2")
            nc.tensor.matmul(ps2[:, R:2 * R], w2_s[:, 128:256], hb_s[:],
                             start=True, stop=True)
            nc.tensor.matmul(ps2[:, 0:R], w2_s[:, 0:128], hb_s[:],
                             start=True, stop=True)
            mu = ps2[:, 0:R]

            # ---- iv = exp(-tanh(z)); exp's accum gives A = sum_r iv ----
            th_s = sb.tile([128, R], F32, tag="th")
            nc.scalar.activation(out=th_s[:], in_=ps2[:, R:2 * R],
                                 func=AF.Tanh, scale=-1.0)
            iv_s = sb.tile([128, R], F32, tag="iv")
            nc.scalar.activation(out=iv_s[:], in_=th_s[:], func=AF.Exp,
                                 accum_out=outv[:, 0:1])

            # ---- products + row-sums on DVE ----
            t1_s = sb.tile([128, R], F32, tag="t1")
            nc.vector.tensor_mul(t1_s[:], mu, iv_s[:])
            t2_s = sb.tile([128, R], F32, tag="t2")
            nc.vector.tensor_mul(t2_s[:], t1_s[:], mu)
            u1_s = sb.tile([128, R], F32, tag="u1")
            nc.vector.tensor_mul(u1_s[:], t1_s[:], yT)
            u2_s = sb.tile([128, R], F32, tag="u2")
            nc.vector.tensor_mul(u2_s[:], ysq_s[:], iv_s[:])
            nc.vector.tensor_reduce(out=outv[:, 1:2], in_=t1_s[:],
                                    axis=AX.X, op=ALU.add)
            nc.vector.tensor_reduce(out=outv[:, 2:3], in_=t2_s[:],
                                    axis=AX.X, op=ALU.add)
            nc.vector.tensor_reduce(out=outv[:, 3:4], in_=u1_s[:],
                                    axis=AX.X, op=ALU.add)
            nc.vector.tensor_reduce(out=outv[:, 4:5], in_=u2_s[:],
                                    axis=AX.X, op=ALU.add)

            nc.sync.dma_start(out=out_d[:], in_=outv[:])

    nc.compile()
    return nc


def _get_nc():
    if "nc" not in _CACHE:
        _CACHE["nc"] = _build()
    return _CACHE["nc"]


def _pack(x_samples, y_samples, w1_mu, b1_mu, w2_mu, b2_mu,
          w1_lv, b1_lv, w2_lv, b2_lv):
    f = np.float32
    w1m = np.asarray(w1_mu, f).reshape(XC, 128, HID)
    w1l = np.asarray(w1_lv, f).reshape(XC, 128, HID)
    w1p = np.zeros((XC, 128, WCOL), f)
    w1p[:, :, 0:8] = w1m
    w1p[:, :, 9:17] = w1l

    b1vec = np.zeros((128, 1), f)
    b1vec[0:8, 0] = np.asarray(b1_mu, f)
    b1vec[8, 0] = 1.0
    b1vec[9:17, 0] = np.asarray(b1_lv, f)
    b1vec[17, 0] = 1.0

    w2 = np.zeros((WCOL, 256), f)
    w2[0:8, 0:128] = np.asarray(w2_mu, f)
    w2[8, 0:128] = np.asarray(b2_mu, f)
    w2[9:17, 128:256] = np.asarray(w2_lv, f)
    w2[17, 128:256] = np.asarray(b2_lv, f)

    x = np.asarray(x_samples, f)
    y = np.asarray(y_samples, f)
    in_maps = []
    for c in range(N_CORES):
        xs = x[c * R:(c + 1) * R]                       # [64, 768]
        xT = xs.reshape(R, XC, 128).transpose(1, 2, 0)  # [6, 128, 64]
        chunks = np.empty((XC, 128, CHUNK), f)
        chunks[:, :, 0:WCOL] = w1p
        chunks[:, :, WCOL:CHUNK] = xT
        a1 = np.ascontiguousarray(
            chunks[0:3].transpose(1, 0, 2).reshape(128, 3 * CHUNK))
        a2 = np.ascontiguousarray(
            chunks[3:6].transpose(1, 0, 2).reshape(128, 3 * CHUNK))
        yb = np.empty((128, 65), f)
        yb[:, 0:R] = y[c * R:(c + 1) * R].T
        yb[:, 64:65] = b1vec
        in_maps.append({"a1": a1, "a2": a2, "yb": yb, "w2": w2})
    return in_maps


def kernel(x_samples, y_samples, w1_mu, b1_mu, w2_mu, b2_mu,
           w1_lv, b1_lv, w2_lv, b2_lv, **profile_kwargs):
    from concourse import bass_utils

    in_maps = _pack(x_samples, y_samples, w1_mu, b1_mu, w2_mu, b2_mu,
                    w1_lv, b1_lv, w2_lv, b2_lv)
    nc = _get_nc()
    res = bass_utils.run_bass_kernel_spmd(
        nc, in_maps, core_ids=list(range(N_CORES)), **profile_kwargs
    )
    acc = np.zeros((128, 7), np.float64)
    for m in res.results:
        acc += m["out"].astype(np.float64)
    A, C, Dv, U1v, U2v, S2, M1 = (acc[:, j] for j in range(7))
    D = Dv.sum()
    P = D - 2.0 * U1v.sum() + U2v.sum()
    neg = (S2 @ A - 2.0 * (M1 @ C) + B * D) / (2.0 * B * B)
    total = -P / (2.0 * B) + neg - np.log1p(np.exp(-20.0) / (B - 1.0))
    out = np.array(total, dtype=np.float32)
    if profile_kwargs:
        return out, res
    return out


# revision 8
# speedup vs baseline: 1.4434x; 1.0800x over previous
"""Trainium2 Bass kernel for nn_L1OutUB (L1-out upper bound contrastive loss).

Math: the reference builds a [B,B,B] tensor `inpt[a,i,j] = all_probs[i,j] +
(-20 if a==i else 0)` and logsumexps over `a`.  That logsumexp is exactly
`all_probs[i,j] + log(B-1+e^-20)`, so

    result = mean(positive) - mean(all_probs) - log1p(e^-20 / (B-1))

`sum_j all_probs[i,j]` collapses onto per-column moments of y, and the
-0.5*logvar terms cancel between positive and negative.  Expanding the
positive-branch square as well, every remaining term is a contraction of
per-core row sums:

    A[d]  = sum_i iv[i,d]          C[d]  = sum_i mu[i,d] iv[i,d]
    D     = sum_{i,d} mu^2 iv      U1    = sum_{i,d} mu y iv   (matched y)
    U2    = sum_{i,d} y^2 iv       S2[d] = sum_j y[j,d]^2      M1[d] = sum_j y[j,d]

    P     = D - 2 U1 + U2          (positive-branch quadratic)
    result = -(P/2B) + (1/2B^2)(S2.A - 2 M1.C + B D) - log1p(e^-20/(B-1))

Sharding: rows of x/y across 8 cores (64 rows each); each core emits its
partial vectors [128, 7] = (A, C, Dv, U1v, U2v, S2, M1); the host sums the
8 partials and does three dot products (the "all-reduce").

Device-side structure per core (layout [d=128 partitions, r=64 free]):
  - host pre-transposes x (xT chunks) and y (yT), packs both MLPs' layer-1
    weights into one dense f32r [128,18] lhsT per chunk whose two zero
    columns, via relu(0 + bias=1), manufacture ones-rows that fold the
    layer-2 biases into the matmuls.  float32r keeps each matmul a single
    PE pass (fp32 runs as two) at near-fp32 precision.
  - PE: 6 accumulating L1 matmuls -> relu -> 2 L2 matmuls whose lhsT are
    zero-padded to base partition 0 ([18,128] each); z's matmul goes first
    so tanh starts while mu's matmul still runs.
  - Scalar: relu, tanh(scale=-1), exp (iv = exp(-tanh(z)); logvar cancels).
  - DVE: iv + 4 products live in one [128, 5*64] tile; a single segmented
    tensor_reduce emits A, C, Dv, U1v, U2v at once.  y moments done early.
  - No transposes, no GpSimd, no collectives; one [128,7] output DMA.
"""

import numpy as np

import concourse.bacc as bacc
import concourse.tile as tile
from concourse import mybir

F32 = mybir.dt.float32
F32R = mybir.dt.float32r
AF = mybir.ActivationFunctionType
ALU = mybir.AluOpType
AX = mybir.AxisListType

B, X_DIM, Y_DIM, HID = 512, 768, 128, 8
N_CORES = 8
R = B // N_CORES          # rows per core = 64
XC = X_DIM // 128         # x feature chunks = 6
WCOL = 18                 # packed L1 lhsT cols: 0:8 mu, 8 zero, 9:17 lv, 17 zero
CHUNK = WCOL + R          # per-chunk blob cols = 82

_CACHE = {}


def _build():
    nc = bacc.Bacc("TRN2", target_bir_lowering=False, debug=False,
                   num_devices=N_CORES)

    # a1/a2: 3 chunks each of [w1p_k [128,18] | xT_k [128,64]], bf16
    a1_d = nc.dram_tensor("a1", [128, 3 * CHUNK], F32R, kind="ExternalInput")
    a2_d = nc.dram_tensor("a2", [128, 3 * CHUNK], F32R, kind="ExternalInput")
    # yb: cols 0:64 = yT (this core's y slice, transposed), col 64 = b1vec
    #     (rows 0:8 b1_mu, row 8 = 1.0, rows 9:17 b1_lv, row 17 = 1.0)
    yb_d = nc.dram_tensor("yb", [128, 65], F32, kind="ExternalInput")
    # w2: cols 0:128 = mu block (rows 0:8 w2_mu, row 8 b2_mu, rows 9:18 zero)
    #     cols 128:256 = lv block (rows 0:9 zero, rows 9:17 w2_lv, row 17 b2_lv)
    w2_d = nc.dram_tensor("w2", [WCOL, 256], F32R, kind="ExternalInput")
    # out columns: A, C, Dv, U1v, U2v, S2, M1
    out_d = nc.dram_tensor("out", [128, 7], F32, kind="ExternalOutput")

    with tile.TileContext(nc) as tc:
        with (
            tc.tile_pool(name="sb", bufs=1) as sb,
            tc.tile_pool(name="ps", bufs=1, space="PSUM") as ps,
        ):
            # ---- loads: x-bearing blobs first on both rings ----
            a1_s = sb.tile([128, 3 * CHUNK], F32R, tag="a1")
            nc.sync.dma_start(out=a1_s[:], in_=a1_d[:])
            a2_s = sb.tile([128, 3 * CHUNK], F32R, tag="a2")
            nc.scalar.dma_start(out=a2_s[:], in_=a2_d[:])
            yb_s = sb.tile([128, 65], F32, tag="yb")
            nc.sync.dma_start(out=yb_s[:], in_=yb_d[:])
            w2_s = sb.tile([WCOL, 256], F32R, tag="w2")
            nc.scalar.dma_start(out=w2_s[:], in_=w2_d[:])

            yT = yb_s[:, 0:R]
            outv = sb.tile([128, 7], F32, tag="outv")
            ysq_s = sb.tile([128, R], F32, tag="ysq")

            # ---- early y moments (only need this core's slice) ----
            nc.vector.tensor_mul(ysq_s[:], yT, yT)
            nc.vector.tensor_reduce(out=outv[:, 5:6], in_=ysq_s[:],
                                    axis=AX.X, op=ALU.add)
            nc.vector.tensor_reduce(out=outv[:, 6:7], in_=yT,
                                    axis=AX.X, op=ALU.add)

            # ---- L1 (both nets fused): ps1 = w1p.T @ xT over 6 chunks ----
            ps1 = ps.tile([WCOL, R], F32, tag="ps1")
            for k in range(XC):
                src = a1_s if k < 3 else a2_s
                j = (k % 3) * CHUNK
                nc.tensor.matmul(ps1[:], src[:, j:j + WCOL],
                                 src[:, j + WCOL:j + CHUNK],
                                 start=(k == 0), stop=(k == XC - 1))
            hb_s = sb.tile([WCOL, R], F32R, tag="hb")
            nc.scalar.activation(out=hb_s[:], in_=ps1[:], func=AF.Relu,
                                 bias=yb_s[0:WCOL, 64:65])

            # ---- L2: z first (tanh is the long pole), then mu ----
            ps2l = ps.tile([128, R], F32, tag="ps2l")
            ps2m = ps.tile([128, R], F32, tag="ps2m")
            nc.tensor.matmul(ps2l[:], w2_s[:, 128:256], hb_s[:],
                             start=True, stop=True)
            nc.tensor.matmul(ps2m[:], w2_s[:, 0:128], hb_s[:],
                             start=True, stop=True)
            mu = ps2m[:]

            # ---- iv = exp(-tanh(z)) straight into the products tile ----
            prods = sb.tile([128, 5, R], F32, tag="prods")
            iv = prods[:, 0, :]
            th_s = sb.tile([128, R], F32, tag="th")
            nc.scalar.activation(out=th_s[:], in_=ps2l[:],
                                 func=AF.Tanh, scale=-1.0)
            nc.scalar.activation(out=iv, in_=th_s[:], func=AF.Exp)

            # ---- products on DVE; one segmented reduce -> A,C,Dv,U1v,U2v ----
            t1 = prods[:, 1, :]
            nc.vector.tensor_mul(t1, mu, iv)
            nc.vector.tensor_mul(prods[:, 2, :], t1, mu)
            nc.vector.tensor_mul(prods[:, 3, :], t1, yT)
            nc.vector.tensor_mul(prods[:, 4, :], ysq_s[:], iv)
            nc.vector.tensor_reduce(out=outv[:, 0:5], in_=prods[:],
                                    axis=AX.X, op=ALU.add)

            nc.sync.dma_start(out=out_d[:], in_=outv[:])

    nc.compile()
    return nc


def _get_nc():
    if "nc" not in _CACHE:
        _CACHE["nc"] = _build()
    return _CACHE["nc"]


def _pack(x_samples, y_samples, w1_mu, b1_mu, w2_mu, b2_mu,
          w1_lv, b1_lv, w2_lv, b2_lv):
    f = np.float32
    w1m = np.asarray(w1_mu, f).reshape(XC, 128, HID)
    w1l = np.asarray(w1_lv, f).reshape(XC, 128, HID)
    w1p = np.zeros((XC, 128, WCOL), f)
    w1p[:, :, 0:8] = w1m
    w1p[:, :, 9:17] = w1l

    b1vec = np.zeros((128, 1), f)
    b1vec[0:8, 0] = np.asarray(b1_mu, f)
    b1vec[8, 0] = 1.0
    b1vec[9:17, 0] = np.asarray(b1_lv, f)
    b1vec[17, 0] = 1.0

    w2 = np.zeros((WCOL, 256), f)
    w2[0:8, 0:128] = np.asarray(w2_mu, f)
    w2[8, 0:128] = np.asarray(b2_mu, f)
    w2[9:17, 128:256] = np.asarray(w2_lv, f)
    w2[17, 128:256] = np.asarray(b2_lv, f)

    x = np.asarray(x_samples, f)
    y = np.asarray(y_samples, f)
    in_maps = []
    for c in range(N_CORES):
        xs = x[c * R:(c + 1) * R]                       # [64, 768]
        xT = xs.reshape(R, XC, 128).transpose(1, 2, 0)  # [6, 128, 64]
        chunks = np.empty((XC, 128, CHUNK), f)
        chunks[:, :, 0:WCOL] = w1p
        chunks[:, :, WCOL:CHUNK] = xT
        a1 = np.ascontiguousarray(
            chunks[0:3].transpose(1, 0, 2).reshape(128, 3 * CHUNK))
        a2 = np.ascontiguousarray(
            chunks[3:6].transpose(1, 0, 2).reshape(128, 3 * CHUNK))
        yb = np.empty((128, 65), f)
        yb[:, 0:R] = y[c * R:(c + 1) * R].T
        yb[:, 64:65] = b1vec
        in_maps.append({"a1": a1, "a2": a2, "yb": yb, "w2": w2})
    return in_maps


def kernel(x_samples, y_samples, w1_mu, b1_mu, w2_mu, b2_mu,
           w1_lv, b1_lv, w2_lv, b2_lv, **profile_kwargs):
    from concourse import bass_utils

    in_maps = _pack(x_samples, y_samples, w1_mu, b1_mu, w2_mu, b2_mu,
                    w1_lv, b1_lv, w2_lv, b2_lv)
    nc = _get_nc()
    res = bass_utils.run_bass_kernel_spmd(
        nc, in_maps, core_ids=list(range(N_CORES)), **profile_kwargs
    )
    acc = np.zeros((128, 7), np.float64)
    for m in res.results:
        acc += m["out"].astype(np.float64)
    A, C, Dv, U1v, U2v, S2, M1 = (acc[:, j] for j in range(7))
    D = Dv.sum()
    P = D - 2.0 * U1v.sum() + U2v.sum()
    neg = (S2 @ A - 2.0 * (M1 @ C) + B * D) / (2.0 * B * B)
    total = -P / (2.0 * B) + neg - np.log1p(np.exp(-20.0) / (B - 1.0))
    out = np.array(total, dtype=np.float32)
    if profile_kwargs:
        return out, res
    return out
